# revision 8
# baseline (speedup 1.0000x reference)
"""MoE (top-2 of 8 experts + shared expert) Trainium2 Bass kernel.

Strategy (expert-parallel, host-prepped routing):
  - Router (sigmoid gate + top-2) is tiny (0.27 GFLOP) and runs on the host
    in fp32; it also produces the token->expert gather lists.
  - Core c computes expert c's SwiGLU FFN densely over the tokens routed to
    it (padded to the max per-expert count), plus the shared-expert FFN over
    the token shard [c*1024, (c+1)*1024).
  - All matmuls run in float32r (full PE rate, ~1.5e-4 rel err).
  - Host scatter-adds the per-expert outputs (scaled by the combine weights)
    and the shared outputs into the final [8192, 2048] result.

Everything on-device is laid out feature-major ("K on partitions") so the
x @ W.T chains need no on-chip transposes:
  stage1:  h1T[m,:] = sum_k w1T[k, m].T @ xT[k, :]   (PSUM accum over k)
  g = silu(h1T) * h3T                                 (ACT + DVE)
  stage2:  yT[md,:] = sum_kh w2T[kh, md].T @ gT[kh,:]
"""

import os
import sys

for _p in ("/opt/trn_rl_repo", "/root/.axon_site/_ro/trn_rl_repo"):
    if os.path.isdir(_p) and _p not in sys.path:
        sys.path.insert(0, _p)

import numpy as np

import concourse.bass as bass  # noqa: F401
import concourse.mybir as mybir
import concourse.tile as tile
from concourse import bacc
from concourse.bass_utils import run_bass_kernel_spmd

# Problem constants (hardcoded per spec)
N_TOK = 8192
D = 2048
H = 1408
E = 8
TOP_K = 2
ROUTE_SCALE = 1.0
P = 128
KD = D // P    # 16 k-tiles over D
MH = H // P    # 11 m-tiles over H
MD = D // P    # 16 m-tiles over D (stage 2 out)
SHARD = N_TOK // E  # 1024 shared-expert tokens per core

MAX_CHUNK = 1152  # tokens per weight-stream chunk (SBUF budget)

F32 = mybir.dt.float32
F32R = mybir.dt.float32r
SILU = mybir.ActivationFunctionType.Silu

LAST_RESULTS = None  # BassKernelResults of the most recent run (for test.py)


def _chunks(T):
    """Split T (multiple of 128) into chunks of <=MAX_CHUNK, multiples of 128."""
    n = -(-T // MAX_CHUNK)
    base = T // n
    sizes = []
    rem = T
    for i in range(n):
        left = n - i
        c = min(MAX_CHUNK, -(-rem // left + 127) // 128 * 128) if left > 1 else rem
        c = min(c, rem)
        sizes.append(c)
        rem -= c
    assert sum(sizes) == T and all(s % 128 == 0 for s in sizes), sizes
    return sizes


def _subs(Tc):
    """Split Tc into matmul free-dim slices, preferring all >=256 (f32r fast)."""
    out = []
    rem = Tc
    while rem > 512:
        take = 384 if rem == 640 else 512
        out.append(take)
        rem -= take
    if rem:
        out.append(rem)
    s0 = 0
    res = []
    for s in out:
        res.append((s0, s))
        s0 += s
    return res


def _emit_ffn(nc, pools, x_dram, w1_dram, w3_dram, w2_dram, y_dram, T):
    """Emit one feature-major SwiGLU FFN over T tokens."""
    xpool, wpool, gpool, spool, ypool, psum = pools
    cs = 0
    for Tc in _chunks(T):
        x_tiles = []
        for k in range(KD):
            xt = xpool.tile([P, Tc], F32R, name=f"x{k}")
            nc.sync.dma_start(xt[:], x_dram[k, :, cs:cs + Tc])
            x_tiles.append(xt)
        g_tiles = []
        for m in range(MH):
            w1m = wpool.tile([P, KD * P], F32R, name="w1m")
            nc.sync.dma_start(w1m[:], w1_dram[m])
            w3m = wpool.tile([P, KD * P], F32R, name="w3m")
            nc.sync.dma_start(w3m[:], w3_dram[m])
            gm = gpool.tile([P, Tc], F32R, name=f"g{m}")
            for s0, sl in _subs(Tc):
                ps1 = psum.tile([P, 512], F32, name="ps1")[:, :sl]
                ps3 = psum.tile([P, 512], F32, name="ps3")[:, :sl]
                for k in range(KD):
                    nc.tensor.matmul(
                        ps1, w1m[:, k * P:(k + 1) * P], x_tiles[k][:, s0:s0 + sl],
                        start=(k == 0), stop=(k == KD - 1),
                    )
                for k in range(KD):
                    nc.tensor.matmul(
                        ps3, w3m[:, k * P:(k + 1) * P], x_tiles[k][:, s0:s0 + sl],
                        start=(k == 0), stop=(k == KD - 1),
                    )
                st = spool.tile([P, 512], F32, name="silu")[:, :sl]
                nc.scalar.activation(st, ps1, SILU)
                nc.vector.tensor_mul(gm[:, s0:s0 + sl], st, ps3)
            g_tiles.append(gm)
        for md in range(MD):
            w2m = wpool.tile([P, MH * P], F32R, name="w2m")
            nc.sync.dma_start(w2m[:], w2_dram[md])
            ym = ypool.tile([P, Tc], F32, name="ym")
            for s0, sl in _subs(Tc):
                psy = psum.tile([P, 512], F32, name="psy")[:, :sl]
                for kh in range(MH):
                    nc.tensor.matmul(
                        psy, w2m[:, kh * P:(kh + 1) * P], g_tiles[kh][:, s0:s0 + sl],
                        start=(kh == 0), stop=(kh == MH - 1),
                    )
                nc.scalar.copy(ym[:, s0:s0 + sl], psy)
            nc.sync.dma_start(y_dram[md, :, cs:cs + Tc], ym[:])
        cs += Tc


def _build_program(c_cap, loop_reps=1):
    nc = bacc.Bacc("TRN2", target_bir_lowering=False, debug=False, num_devices=E)
    xe = nc.dram_tensor("xe", [KD, P, c_cap], F32R, kind="ExternalInput").ap()
    xs = nc.dram_tensor("xs", [KD, P, SHARD], F32R, kind="ExternalInput").ap()
    w1s = nc.dram_tensor("w1s", [MH, P, KD * P], F32R, kind="ExternalInput").ap()
    w3s = nc.dram_tensor("w3s", [MH, P, KD * P], F32R, kind="ExternalInput").ap()
    w2s = nc.dram_tensor("w2s", [MD, P, MH * P], F32R, kind="ExternalInput").ap()
    sw1s = nc.dram_tensor("sw1s", [MH, P, KD * P], F32R, kind="ExternalInput").ap()
    sw3s = nc.dram_tensor("sw3s", [MH, P, KD * P], F32R, kind="ExternalInput").ap()
    sw2s = nc.dram_tensor("sw2s", [MD, P, MH * P], F32R, kind="ExternalInput").ap()
    ye = nc.dram_tensor("ye", [MD, P, c_cap], F32, kind="ExternalOutput").ap()
    ys = nc.dram_tensor("ys", [MD, P, SHARD], F32, kind="ExternalOutput").ap()

    with tile.TileContext(nc) as tc:
        with tc.tile_pool(name="xpool", bufs=1) as xpool, \
             tc.tile_pool(name="wpool", bufs=2) as wpool, \
             tc.tile_pool(name="gpool", bufs=1) as gpool, \
             tc.tile_pool(name="spool", bufs=2) as spool, \
             tc.tile_pool(name="ypool", bufs=2) as ypool, \
             tc.tile_pool(name="psum", bufs=2, space="PSUM") as psum:
            pools = (xpool, wpool, gpool, spool, ypool, psum)
            if loop_reps > 1:
                with tc.For_i(0, loop_reps, 1):
                    _emit_ffn(nc, pools, xe, w1s, w3s, w2s, ye, c_cap)
                    _emit_ffn(nc, pools, xs, sw1s, sw3s, sw2s, ys, SHARD)
            else:
                _emit_ffn(nc, pools, xe, w1s, w3s, w2s, ye, c_cap)
                _emit_ffn(nc, pools, xs, sw1s, sw3s, sw2s, ys, SHARD)
    nc.compile()
    return nc


def _tile_w13(w):
    # [H, D] -> [MH, P, KD*P] with slab[m, p, k*P+j] = w[m*P+j, k*P+p]
    return np.ascontiguousarray(
        w.reshape(MH, P, KD, P).transpose(0, 3, 2, 1).reshape(MH, P, KD * P)
    )


def _tile_w2(w):
    # [D, H] -> [MD, P, MH*P] with slab[md, p, kh*P+j] = w[md*P+j, kh*P+p]
    return np.ascontiguousarray(
        w.reshape(MD, P, MH, P).transpose(0, 3, 2, 1).reshape(MD, P, MH * P)
    )


def _tile_x(xt):
    # [T, D] -> [KD, P, T]
    T = xt.shape[0]
    return np.ascontiguousarray(xt.reshape(T, KD, P).transpose(1, 2, 0))


def _untile_y(y):
    # [MD, P, T] -> [T, D]
    return y.transpose(2, 0, 1).reshape(y.shape[2], D)


def prepare(x, gate_w, expert_bias, w1, w2, w3, sw1, sw2, sw3):
    """Host routing + input prep. Returns (nc, in_maps, meta)."""
    x = np.ascontiguousarray(np.asarray(x, dtype=np.float32))
    gate_w = np.asarray(gate_w, dtype=np.float32)
    expert_bias = np.asarray(expert_bias, dtype=np.float32)
    w1 = np.asarray(w1, dtype=np.float32)
    w2 = np.asarray(w2, dtype=np.float32)
    w3 = np.asarray(w3, dtype=np.float32)
    sw1 = np.asarray(sw1, dtype=np.float32)
    sw2 = np.asarray(sw2, dtype=np.float32)
    sw3 = np.asarray(sw3, dtype=np.float32)

    # ---- host router (fp32, matches reference numerics) ----
    logits = x @ gate_w.T  # [N, E] f32
    scores = np.where(
        logits >= 0,
        1.0 / (1.0 + np.exp(-logits, dtype=np.float32)),
        np.exp(logits, dtype=np.float32) / (1.0 + np.exp(logits, dtype=np.float32)),
    ).astype(np.float32)
    biased = scores + expert_bias[None, :]
    i1 = np.argmax(biased, axis=1)
    tmp = biased.copy()
    tmp[np.arange(N_TOK), i1] = -np.inf
    i2 = np.argmax(tmp, axis=1)
    s1 = scores[np.arange(N_TOK), i1]
    s2 = scores[np.arange(N_TOK), i2]
    denom = s1 + s2 + np.float32(1e-20)
    c1 = (s1 / denom * np.float32(ROUTE_SCALE)).astype(np.float32)
    c2 = (s2 / denom * np.float32(ROUTE_SCALE)).astype(np.float32)

    idx_list, cw_list = [], []
    for e in range(E):
        m1 = i1 == e
        m2 = i2 == e
        idx = np.concatenate([np.nonzero(m1)[0], np.nonzero(m2)[0]])
        cw = np.concatenate([c1[m1], c2[m2]]).astype(np.float32)
        idx_list.append(idx)
        cw_list.append(cw)
    counts = [len(i) for i in idx_list]
    c_cap = max(512, -(-max(counts) // 128) * 128)

    # ---- build + compile the SPMD program for this capacity ----
    nc = _build_program(c_cap, loop_reps=int(os.environ.get("MOE_LOOP_REPS", "1")))

    # ---- per-core inputs ----
    in_maps = []
    sw1s = _tile_w13(sw1)
    sw3s = _tile_w13(sw3)
    sw2s = _tile_w2(sw2)
    for c in range(E):
        idx = idx_list[c]
        pad = c_cap - len(idx)
        idx_pad = np.concatenate([idx, np.zeros(pad, dtype=idx.dtype)]) if pad else idx
        xe = x[idx_pad]
        in_maps.append({
            "xe": _tile_x(xe),
            "xs": _tile_x(x[c * SHARD:(c + 1) * SHARD]),
            "w1s": _tile_w13(w1[c]),
            "w3s": _tile_w13(w3[c]),
            "w2s": _tile_w2(w2[c]),
            "sw1s": sw1s,
            "sw3s": sw3s,
            "sw2s": sw2s,
        })

    meta = (idx_list, cw_list, counts)
    return nc, in_maps, meta


def combine(meta, results):
    """Scatter-add per-core outputs into the final [N, D] array."""
    idx_list, cw_list, counts = meta
    out = np.zeros((N_TOK, D), dtype=np.float32)
    for c in range(E):
        r = results[c]
        cnt = counts[c]
        if cnt:
            y_tok = _untile_y(r["ye"])[:cnt]
            out[idx_list[c]] += cw_list[c][:, None] * y_tok
        out[c * SHARD:(c + 1) * SHARD] += _untile_y(r["ys"])
    return out


def kernel(x, gate_w, expert_bias, w1, w2, w3, sw1, sw2, sw3):
    nc, in_maps, meta = prepare(x, gate_w, expert_bias, w1, w2, w3, sw1, sw2, sw3)
    global LAST_RESULTS
    res = run_bass_kernel_spmd(nc, in_maps, core_ids=list(range(E)))
    LAST_RESULTS = res
    return combine(meta, res.results)
